# revision 14
# baseline (speedup 1.0000x reference)
"""Trainium2 Bass kernel for MinimalRNNCell:  h_t = x_t @ K + h_{t-1} @ R.

Shapes (full): x [64, 512, 512], h0 [64, 512], kernel [512, 512],
recurrent_kernel [512, 512] -> out [64, 512, 512], all float32.

Strategy: data-parallel over batch (8 rows/core on 8 cores). Per core, a
blocked linear scan keeps the 128x128 PE array full despite the tiny
per-core batch:

  - state is kept transposed, h^T [U, cols]: matmul(lhsT=R, rhs=h^T)
    computes (h @ R)^T so each step's output feeds the next directly.
  - T=512 is split into NB=16 blocks of S=32 steps. All blocks scan in
    parallel as extra state columns (W = 16 blocks * 8 batch = 128 cols
    per matmul -> full PE utilization).
      sweep A: zero-carry block scans  -> per-block final states A_j
      chain:   c_{j+1} = c_j @ R^S + A_j  (16 small sequential steps;
               R^S precomputed on host in float64)
      phase 3: carry-corrected block scans emit every h_t.
  - xk = x @ K is computed on-device in transposed layout: x tiles are
    DMA'd with partition = (t16, b8) interleaved so a PE transpose lands
    them directly in the scan's column order (col = t*BC + b), making
    every PSUM->SBUF evacuation contiguous.
  - phase-3 outputs are PE-transposed back to natural layout for
    contiguous DMA to DRAM.
"""

import numpy as np

B, T, D, U = 64, 512, 512, 512
NCORES = 8
BC = B // NCORES          # batch rows per core

_BUILD_CACHE = {}


def _block_size(mmdt):
    # f32r matmuls only reach 1 cycle/row with moving dim >= 256, so use
    # more, smaller blocks there (W = T/S * BC state columns per matmul)
    return 16 if mmdt == "f32r" else 32


def _build(reps=1, phases=(1, 2, 3, 4), mmdt="f32"):
    S = _block_size(mmdt)
    NB = T // S               # blocks
    W = NB * BC               # state columns per scan step
    import concourse.mybir as mybir
    import concourse.tile as tile
    from concourse import bacc
    from concourse.bass import ts, ds
    from concourse.masks import make_identity
    from contextlib import nullcontext

    f32 = mybir.dt.float32
    fmm = mybir.dt.float32r if mmdt == "f32r" else f32
    nc = bacc.Bacc("TRN2", target_bir_lowering=False, debug=False,
                   num_devices=NCORES)

    x_d = nc.dram_tensor("x", [BC, T, D], fmm, kind="ExternalInput").ap()
    h0t_d = nc.dram_tensor("h0t", [U, BC], fmm, kind="ExternalInput").ap()
    wk_d = nc.dram_tensor("wk", [D, U], fmm, kind="ExternalInput").ap()
    wr_d = nc.dram_tensor("wr", [U, U], fmm, kind="ExternalInput").ap()
    # stacked carry-hop powers: wps[t] = R^(S*(t+1)), t = 0..3
    wps_d = nc.dram_tensor("wps", [4, U, U], fmm, kind="ExternalInput").ap()
    out_d = nc.dram_tensor("out", [BC, T, U], f32, kind="ExternalOutput").ap()

    KC = D // 128  # contraction chunks (4)
    MC = U // 128  # output-row chunks (4)
    NG = (T * BC) // 512  # phase-1 column groups (8): 512 cols each

    with tile.TileContext(nc) as tc:
        with (
            tc.tile_pool(name="weights", bufs=1) as wpool,
            tc.tile_pool(name="xkt", bufs=1) as xkpool,
            tc.tile_pool(name="xa", bufs=4) as xapool,
            tc.tile_pool(name="xt", bufs=2) as xtpool,
            tc.tile_pool(name="sa", bufs=2) as sapool,
            tc.tile_pool(name="abuf", bufs=1) as apool,
            tc.tile_pool(name="cb", bufs=1) as cbpool,
            tc.tile_pool(name="s3", bufs=3) as s3pool,
            tc.tile_pool(name="ob", bufs=3) as obpool,
        ):
            loop_ctx = tc.For_i(0, reps, 1) if reps > 1 else nullcontext()
            with loop_ctx:
                ident_t = wpool.tile([128, 128], f32, tag="ident",
                                     name="ident")
                make_identity(nc, ident_t[:])
                if fmm != f32:
                    # transpose needs an identity matching the data dtype,
                    # produced by an instruction that rounds to f32r
                    ident_r = wpool.tile([128, 128], fmm, tag="identr",
                                         name="identr")
                    nc.vector.tensor_copy(ident_r[:], ident_t[:])
                    ident = ident_r
                else:
                    ident = ident_t

                kw = [wpool.tile([128, U], fmm, tag=f"kw{k}", name=f"kw{k}")
                      for k in range(KC)]
                rw = [wpool.tile([128, U], fmm, tag=f"rw{k}", name=f"rw{k}")
                      for k in range(KC)]
                pw = [[wpool.tile([128, U], fmm, tag=f"pw{t}_{k}",
                                  name=f"pw{t}_{k}") for k in range(KC)]
                      for t in range(4)]
                for k in range(KC):
                    nc.sync.dma_start(kw[k][:], wk_d[ts(k, 128), :])
                    nc.sync.dma_start(rw[k][:], wr_d[ts(k, 128), :])
                    for t in range(4):
                        nc.sync.dma_start(pw[t][k][:], wps_d[t, ts(k, 128), :])

                xkt = [xkpool.tile([128, T * BC], fmm, tag=f"xkt{m}",
                                   name=f"xkt{m}") for m in range(MC)]

                def xkt_cols(m, i):
                    # columns (j, b) of xk^T chunk m for in-block step i
                    return (xkt[m][:]
                            .rearrange("p (j s b) -> p j s b", s=S, b=BC)
                            [:, :, i, :])

                # x viewed so one [128, D] tile has partition = (t16, b8):
                # t = g*64 + sub*16 + tl  ->  xkt column g*512+sub*128+tl*8+b
                x_v = x_d.rearrange("b (g s tl) d -> g s tl b d", g=NG, s=4)

                # ---- phase 1: xk^T = (x @ K)^T, col = t*BC + b ----
                if 1 in phases:
                    with tc.tile_pool(name="ps1", bufs=1, space="PSUM") as ps1:
                        for g in range(NG):
                            tp = [ps1.tile([128, 512], fmm, tag=f"tp{d4}",
                                           name=f"tp{d4}") for d4 in range(KC)]
                            for sub in range(4):
                                xa = xapool.tile([128, D], fmm, tag="xa",
                                                 name="xa")
                                nc.sync.dma_start(xa[:], x_v[g, sub])
                                for d4 in range(KC):
                                    nc.tensor.transpose(
                                        tp[d4][:, ts(sub, 128)],
                                        xa[:, ts(d4, 128)], ident[:])
                            xt = [xtpool.tile([128, 512], fmm, tag=f"xt{d4}",
                                              name=f"xt{d4}")
                                  for d4 in range(KC)]
                            for d4 in range(KC):
                                nc.vector.tensor_copy(xt[d4][:], tp[d4][:])
                            for m in range(MC):
                                mm = ps1.tile([128, 512], f32, tag=f"mm{m}",
                                              name=f"mm{m}")
                                for k in range(KC):
                                    nc.tensor.matmul(
                                        mm[:], kw[k][:, ts(m, 128)], xt[k][:],
                                        start=(k == 0), stop=(k == KC - 1))
                                nc.vector.tensor_copy(
                                    xkt[m][:, ts(g, 512)], mm[:])

                with (
                    tc.tile_pool(name="ps2", bufs=1, space="PSUM") as ps2,
                    tc.tile_pool(name="pst", bufs=2, space="PSUM") as pst,
                ):
                    # ---- sweep A: zero-carry block scans (batched) ----
                    if 2 in phases:
                        st = [sapool.tile([128, W], fmm, tag=f"sa{m}",
                                          name=f"sa{m}") for m in range(MC)]
                        for m in range(MC):
                            nc.vector.tensor_copy(st[m][:], xkt_cols(m, 0))
                        for i in range(1, S):
                            ps = [ps2.tile([128, W], f32, tag=f"ps{m}",
                                           name=f"ps{m}") for m in range(MC)]
                            for m in range(MC):
                                for k in range(KC):
                                    nc.tensor.matmul(
                                        ps[m][:], rw[k][:, ts(m, 128)],
                                        st[k][:],
                                        start=(k == 0), stop=(k == KC - 1))
                            if i < S - 1:
                                new = [sapool.tile([128, W], fmm, tag=f"sa{m}",
                                                   name=f"sa{m}")
                                       for m in range(MC)]
                            else:
                                new = [apool.tile([128, W], fmm, tag=f"A{m}",
                                                  name=f"A{m}")
                                       for m in range(MC)]
                            for m in range(MC):
                                nc.vector.tensor_add(new[m][:], ps[m][:],
                                                     xkt_cols(m, i))
                            st = new
                        A = st  # per-block final states

                    # ---- chain: c_0 = h0^T; c_{j+1} = c_j @ R^S + A_j ----
                    # computed at stride 4 to shorten the serial part:
                    #   D_j      = sum_s A_{j+s} @ P^{3-s}          (batched)
                    #   c_{j+4}  = c_j @ P^4 + D_j                  (serial)
                    #   c_{j+t}  = c_j @ P^t + sum_{s<t} A_{j+s} @ P^{t-1-s}
                    #                                               (batched)
                    # where P = R^S and pw[t] holds P^(t+1).
                    if 3 in phases:
                        NQ = NB // 4  # stride-4 groups
                        GW = NQ * BC  # batched group width

                        def gcols(t_ap, off):
                            # cols (j = 4q + off, b) for q = 0..NQ-1
                            return (t_ap.rearrange("p (q c) -> p q c",
                                                   c=4 * BC)
                                    [:, :, off * BC:(off + 1) * BC])

                        cb = [cbpool.tile([128, W], fmm, tag=f"cb{m}",
                                          name=f"cb{m}") for m in range(MC)]
                        for m in range(MC):
                            nc.sync.dma_start(cb[m][:, 0:BC],
                                              h0t_d[ts(m, 128), :])
                        # batched D terms
                        Dt = [apool.tile([128, GW], fmm, tag=f"D{m}",
                                         name=f"D{m}") for m in range(MC)]
                        for m in range(MC):
                            dj = ps2.tile([128, GW], f32, tag=f"ps{m}",
                                          name=f"ps{m}")
                            for s in range(3):
                                for k in range(KC):
                                    nc.tensor.matmul(
                                        dj[:], pw[2 - s][k][:, ts(m, 128)],
                                        gcols(A[k][:], s),
                                        start=(s == 0 and k == 0),
                                        stop=(s == 2 and k == KC - 1))
                            nc.vector.tensor_add(Dt[m][:], dj[:],
                                                 gcols(A[m][:], 3))
                        # serial rounds: c_{4q+4} = c_{4q} @ P^4 + D_{4q}
                        for q in range(NQ - 1):
                            pc = [ps2.tile([128, BC], f32, tag=f"ps{m}",
                                           name=f"ps{m}") for m in range(MC)]
                            for m in range(MC):
                                for k in range(KC):
                                    nc.tensor.matmul(
                                        pc[m][:], pw[3][k][:, ts(m, 128)],
                                        cb[k][:, ds(4 * q * BC, BC)],
                                        start=(k == 0), stop=(k == KC - 1))
                            for m in range(MC):
                                nc.vector.tensor_add(
                                    cb[m][:, ds((4 * q + 4) * BC, BC)],
                                    pc[m][:], Dt[m][:, ds(q * BC, BC)])
                        # batched fill of the in-between carries
                        for off in range(1, 4):
                            for m in range(MC):
                                fp = ps2.tile([128, GW], f32, tag=f"ps{m}",
                                              name=f"ps{m}")
                                terms = [(pw[off - 1],
                                          lambda k: gcols(cb[k][:], 0))]
                                for s in range(off - 1):
                                    terms.append(
                                        (pw[off - 2 - s],
                                         lambda k, s=s: gcols(A[k][:], s)))
                                nmm = len(terms) * KC
                                idx = 0
                                for pwt, rhs_of in terms:
                                    for k in range(KC):
                                        nc.tensor.matmul(
                                            fp[:], pwt[k][:, ts(m, 128)],
                                            rhs_of(k),
                                            start=(idx == 0),
                                            stop=(idx == nmm - 1))
                                        idx += 1
                                nc.vector.tensor_add(
                                    gcols(cb[m][:], off), fp[:],
                                    gcols(A[m][:], off - 1))

                    # ---- phase 3: carry-corrected scans, emit all h_t ----
                    if 4 in phases:
                        prev = cb
                        for i in range(S):
                            ps = [ps2.tile([128, W], f32, tag=f"ps{m}",
                                           name=f"ps{m}") for m in range(MC)]
                            for m in range(MC):
                                for k in range(KC):
                                    nc.tensor.matmul(
                                        ps[m][:], rw[k][:, ts(m, 128)],
                                        prev[k][:],
                                        start=(k == 0), stop=(k == KC - 1))
                            cur = [s3pool.tile([128, W], fmm, tag=f"s3{m}",
                                               name=f"s3{m}")
                                   for m in range(MC)]
                            for m in range(MC):
                                nc.vector.tensor_add(cur[m][:], ps[m][:],
                                                     xkt_cols(m, i))
                            prev = cur
                            JH = 128 // BC  # blocks per 128-col half
                            for h in range(W // 128):
                                pt = pst.tile([128, U], fmm, tag=f"pt{h}",
                                              name=f"pt{h}")
                                for m in range(MC):
                                    nc.tensor.transpose(
                                        pt[:, ts(m, 128)],
                                        cur[m][:, ts(h, 128)], ident[:])
                                ob = obpool.tile([128, U], f32, tag=f"ob{h}",
                                                 name=f"ob{h}")
                                nc.vector.tensor_copy(ob[:], pt[:])
                                dst = (out_d
                                       .rearrange("b (j s) u -> j b s u", s=S)
                                       [ds(h * JH, JH), :, i, :])
                                nc.sync.dma_start(dst, ob[:])

    nc.finalize()
    return nc


def _get_nc(reps=1, phases=(1, 2, 3, 4), mmdt="f32"):
    key = (reps, tuple(phases), mmdt)
    if key not in _BUILD_CACHE:
        _BUILD_CACHE[key] = _build(reps, phases, mmdt)
    return _BUILD_CACHE[key]


def kernel(x, h0, kernel, recurrent_kernel, _reps=1, _time_only=False,
           _phases=(1, 2, 3, 4), _mmdt="f32r"):
    from concourse.bass_utils import run_bass_kernel_spmd

    x = np.ascontiguousarray(x, np.float32)
    h0 = np.ascontiguousarray(h0, np.float32)
    wk = np.ascontiguousarray(kernel, np.float32)
    wr = np.ascontiguousarray(recurrent_kernel, np.float32)
    # carry-hop powers R^(S*t), t=1..4, in float64 on host (more accurate
    # than a device-side fp32 chain)
    p1 = np.linalg.matrix_power(wr.astype(np.float64), _block_size(_mmdt))
    wps = np.stack([p1, p1 @ p1, p1 @ p1 @ p1,
                    (p1 @ p1) @ (p1 @ p1)]).astype(np.float32)

    nc = _get_nc(_reps, _phases, _mmdt)
    in_maps = []
    for c in range(NCORES):
        sl = slice(c * BC, (c + 1) * BC)
        in_maps.append({
            "x": np.ascontiguousarray(x[sl]),
            "h0t": np.ascontiguousarray(h0[sl].T),
            "wk": wk, "wr": wr, "wps": wps,
        })
    res = run_bass_kernel_spmd(nc, in_maps, core_ids=list(range(NCORES)))
    if _time_only:
        return None
    return np.concatenate([res.results[c]["out"] for c in range(NCORES)],
                          axis=0)
